# revision 16
# baseline (speedup 1.0000x reference)
"""Trainium2 Bass kernel for nn_H_ATT (GatedTrans pair-attention block).

Math (per example):
  HE = tanh(hist@W_hy+b_hy) * lrelu(hist@W_hg+b_hg)      [R, H]
  QE = tanh(ques@W_qy+b_qy) * lrelu(ques@W_qg+b_qg)      [R, H]
  num[q,h]  = sum_k QE[q,k]*W_att[k]*HE[h,k]
  den[q,h]  = sqrt(sum_k QE[q,k]^2 * HE[h,k]^2)
  s = num / max(den, eps)          (b_att cancels in softmax)
  att = causal_softmax(s)          (softmax*tril/renorm == masked softmax)
  feat = att @ hist                 [R, 2H]

Sharding: pure data parallel, 8 examples per core on 8 NeuronCores.

Perf structure (fp8 mode, default):
- All DRAM operand layouts are partition-major so every DMA reads long
  contiguous (4KB/partition) runs.
- The 4 big embedding GEMMs run as fp8e4 DoubleRow matmuls (2 k-tiles
  per instruction -> 2x PE throughput, 109ns/instr measured warm).
  Weights are pre-scaled by 64 on the host so W*64 ~ N(0,1.4) sits in
  e4m3's normal range; the scale is undone exactly: tanh gets
  scale=1/64, leaky_relu is positively homogeneous so the 64x rides
  through and cancels against watt/64^2 in num and scale=1/64 inside
  the squares for den.
- Weight-tile DMAs alternate between the two HWDGE rings (sync/SP and
  scalar/ACT) so descriptor generation pipelines; consts are merged
  into single transfers.
- The causal mask is -1e30 on ALL cross-example positions, so softmax
  is one full 128x128 exp + row reduce; the 1/rowsum is folded into
  the feat PSUM->SBUF copies (rows of feat PSUM are q).
- hist for the final feat matmul and the output are bf16.
"""

import numpy as np
import ml_dtypes

import bass_rust
import concourse.bass as bass
import concourse.mybir as mybir
import concourse.tile as tile
from concourse.vector_clock import ScopedClock

# ---------------------------------------------------------------------------
# Workaround: this walrus build accepts only ONE semaphore wait on an SP
# Drain, but TileContext's tail drain carries one wait per live semaphore.
# Split them across a chain of drains.
# ---------------------------------------------------------------------------


def _patched_drain_and_barrier(self, tick_clock, wait_clock):
    nc = self.nc
    drain_inst = nc.sync.drain()
    wait_clock.add_sem_waits(
        drain_inst.ins, ScopedClock({None: tick_clock.global_clock})
    )
    waits = list(drain_inst.ins.sync_info.on_wait)
    if len(waits) > 1:
        drain_inst.ins.sync_info = bass_rust.SyncInfo(
            on_wait=waits[:1], on_update=list(drain_inst.ins.sync_info.on_update)
        )
        for i in range(1, len(waits)):
            extra = nc.sync.drain()
            extra.ins.sync_info = bass_rust.SyncInfo(
                on_wait=waits[i : i + 1], on_update=[]
            )
    nc.all_engine_barrier()
    assert self.sems is not None
    popped = nc._tile_sem_poison_stack.pop()
    assert popped is self._sem_poison
    nc.clear_and_free_semaphores(list(self.sems.allocated().values()))
    nc.all_engine_barrier()


tile.TileContext._drain_and_barrier = _patched_drain_and_barrier


def _split_multi_waits(nc):
    """This walrus build accepts at most one semaphore wait per instruction.
    Hoist extra waits onto standalone EventSemaphore instructions inserted
    just before the owning instruction in the same engine's stream."""
    uid = [0]
    for f in nc.m.functions:
        for bb in f.blocks:
            out = []
            for inst in bb.instructions:
                si = inst.sync_info
                if si is not None and len(si.on_wait) > 1:
                    waits = list(si.on_wait)
                    for w in waits[:-1]:
                        nop = mybir.InstEventSemaphore(
                            name=f"I-waitsplit-{uid[0]}", ins=[], outs=[]
                        )
                        uid[0] += 1
                        nop.engine = inst.engine
                        nop.sync_info = bass_rust.SyncInfo(
                            on_wait=[w], on_update=[]
                        )
                        out.append(nop)
                    inst.sync_info = bass_rust.SyncInfo(
                        on_wait=[waits[-1]], on_update=list(si.on_update)
                    )
                out.append(inst)
            bb.instructions[:] = out

# ---------------------------------------------------------------------------

B, R, H, IN = 64, 32, 1024, 2048
NCORES = 8
BL = B // NCORES  # examples per core
BR = BL * R  # 256 rows per core
KC = IN // 128  # 16 contraction chunks
MC = H // 128  # 8 h chunks
NEG = -1.0e30
WSCALE = 64.0  # fp8 weight pre-scale (power of two)

F32 = mybir.dt.float32
BF16 = mybir.dt.bfloat16


def build_program(mode="fp8", zero_bias=True):
    """Build the per-core Bass program. mode selects the dtype of the
    big-GEMM operands (weights + transposed activations):
    fp8 (DoubleRow, weights pre-scaled), bf16, or f32r."""
    if mode == "fp8":
        xdt = mybir.dt.float8e4
        step = 2
        pmode = mybir.MatmulPerfMode.DoubleRow
        sinv = 1.0 / WSCALE
    else:
        xdt = mybir.dt.float32r if mode == "f32r" else BF16
        step = 1
        pmode = None
        sinv = 1.0

    nc = bass.Bass()
    qt_d = nc.dram_tensor("qt", [128, KC, BR], xdt, kind="ExternalInput")
    ht_d = nc.dram_tensor("ht", [128, KC, BR], xdt, kind="ExternalInput")
    hn_d = nc.dram_tensor("hn", [128, 2, IN], BF16, kind="ExternalInput")
    wh_d = nc.dram_tensor("wh", [MC, 128, 2, KC, 128], xdt, kind="ExternalInput")
    wq_d = nc.dram_tensor("wq", [MC, 128, 2, KC, 128], xdt, kind="ExternalInput")
    # [bqy, bqg, bhy, bhg, watt] stacked -> one DMA
    bw_d = nc.dram_tensor("bw", [128, 5, MC], F32, kind="ExternalInput")
    # [mask | ident] -> one DMA
    mi_d = nc.dram_tensor("mi", [128, 256], F32, kind="ExternalInput")
    feat_d = nc.dram_tensor("feat", [2, 128, IN], BF16, kind="ExternalOutput")

    ACT = mybir.ActivationFunctionType

    with tile.TileContext(nc) as tc:
        with (
            tc.tile_pool(name="sb", bufs=1) as big,
            tc.tile_pool(name="ps", bufs=1, space="PSUM") as psp,
        ):
            sm = big
            wts = big
            tmp = big
            pse = psp
            psnd = psp
            psf = psp
            # consts: one small transfer on the scalar ring; the sync ring
            # carries the whole weight stream
            bw = sm.tile([128, 5, MC], F32, tag="bw")
            nc.scalar.dma_start(bw[:], bw_d[:])
            # ques-transposed activations: first compute dependency; split
            # across the two rings so the first matmul starts sooner
            qt = big.tile([128, KC, BR], xdt, tag="qt")
            nc.sync.dma_start(qt[:, 0:8, :], qt_d[:, 0:8, :])
            nc.scalar.dma_start(qt[:, 8:16, :], qt_d[:, 8:16, :])

            EDT = BF16
            he = big.tile([128, MC, BR], EDT, tag="he")
            he2 = big.tile([128, MC, BR], EDT, tag="he2")
            qew = big.tile([128, MC, BR], EDT, tag="qew")
            qe2 = big.tile([128, MC, BR], EDT, tag="qe2")

            num_ps = [
                psnd.tile([128, 128], F32, name=f"num{g}", tag=f"num{g}")[:]
                for g in range(2)
            ]
            den_ps = [
                psnd.tile([128, 128], F32, name=f"den{g}", tag=f"den{g}")[:]
                for g in range(2)
            ]

            def gated(xt, w_dram, iy, ig, m, split=False):
                """One contiguous y+g weight DMA (sync ring); big GEMM pair.
                Returns (ty, tg) [128, BR]: ty = tanh branch, tg = the
                (64x-scaled in fp8 mode) leaky_relu branch."""
                wt = wts.tile([128, 2, KC, 128], xdt, tag="wt", bufs=8)
                if split:
                    # y half first so the first matmuls start half a
                    # transfer earlier
                    nc.sync.dma_start(wt[:, 0], w_dram[m, :, 0])
                    nc.sync.dma_start(wt[:, 1], w_dram[m, :, 1])
                else:
                    nc.sync.dma_start(wt[:], w_dram[m])
                ps = pse.tile([128, 2 * BR], F32, tag="ps", bufs=2)
                psy, psg = ps[:, 0:BR], ps[:, BR : 2 * BR]
                for k in range(0, KC, step):
                    nc.tensor.matmul(
                        psy,
                        wt[:, 0, k : k + step, :] if step == 2 else wt[:, 0, k, :],
                        xt[:, k : k + step, :] if step == 2 else xt[:, k, :],
                        start=(k == 0), stop=(k + step == KC),
                        perf_mode=pmode,
                    )
                for k in range(0, KC, step):
                    nc.tensor.matmul(
                        psg,
                        wt[:, 1, k : k + step, :] if step == 2 else wt[:, 1, k, :],
                        xt[:, k : k + step, :] if step == 2 else xt[:, k, :],
                        start=(k == 0), stop=(k + step == KC),
                        perf_mode=pmode,
                    )
                ty = tmp.tile([128, BR], F32, tag="ty", bufs=3)
                nc.scalar.activation(
                    ty[:], psy, ACT.Tanh, bias=bw[:, iy, m : m + 1], scale=sinv
                )
                # leaky_relu(s*x) = s*leaky_relu(x): the 64x rides along
                tg = tmp.tile([128, BR], F32, tag="tg", bufs=3)
                nc.scalar.activation(
                    tg[:], psg, ACT.Lrelu, bias=bw[:, ig, m : m + 1], alpha=0.01
                )
                return ty, tg

            # ques embeddings (first: only needs qt + wq)
            for m in range(MC):
                ty, tg = gated(qt, wq_d, 0, 1, m, split=(m == 0))
                # qew = ty * (watt/64^2) * tg_scaled  -> qew_true/64
                nc.vector.scalar_tensor_tensor(
                    qew[:, m, :], ty[:], bw[:, 4, m : m + 1], tg[:],
                    op0=mybir.AluOpType.mult, op1=mybir.AluOpType.mult,
                )
                qe = tmp.tile([128, BR], F32, tag="qe", bufs=3)
                nc.vector.tensor_mul(qe[:], ty[:], tg[:])
                # (qe_scaled/64)^2 = qe_true^2
                nc.scalar.activation(qe2[:, m, :], qe[:], ACT.Square, scale=sinv)
                if m == 2:
                    # hist inputs on the scalar ring once the early rush
                    # (qt + first weight tiles) has drained
                    ht = big.tile([128, KC, BR], xdt, tag="ht")
                    nc.scalar.dma_start(ht[:], ht_d[:])
                if m == 5:
                    hn = big.tile([128, 2, IN], BF16, tag="hn")
                    nc.scalar.dma_start(hn[:], hn_d[:])
                if m == 6:
                    mi = sm.tile([128, 256], F32, tag="mi")
                    nc.scalar.dma_start(mi[:], mi_d[:])


            # hist embeddings + num/den accumulation per chunk
            for m in range(MC):
                ty, tg = gated(ht, wh_d, 2, 3, m)
                nc.vector.tensor_mul(he[:, m, :], ty[:], tg[:])
                for g in range(2):
                    sl = slice(128 * g, 128 * (g + 1))
                    nc.tensor.matmul(
                        num_ps[g], qew[:, m, sl], he[:, m, sl],
                        start=(m == 0), stop=(m == MC - 1),
                    )
                nc.scalar.activation(he2[:, m, :], he[:, m, :], ACT.Square, scale=sinv)
                for g in range(2):
                    sl = slice(128 * g, 128 * (g + 1))
                    nc.tensor.matmul(
                        den_ps[g], qe2[:, m, sl], he2[:, m, sl],
                        start=(m == 0), stop=(m == MC - 1),
                    )

            # Dummy Sqrt/Exp with no data deps: their ~1.3us PWP table
            # loads run while the PE finishes the num/den accumulation,
            # instead of on the serial tail path below.
            warm = sm.tile([128, 1], F32, tag="warm")
            nc.scalar.activation(warm[:], bw[:, 4, 0:1], ACT.Sqrt)
            warm2 = sm.tile([128, 1], F32, tag="warm2")
            nc.scalar.activation(warm2[:], bw[:, 4, 0:1], ACT.Exp)

            # attention tail + feat: mask is -1e30 off the causal diagonal
            # blocks, so exp of the full tile zeroes cross-example terms and
            # the row sum is the softmax denominator; 1/rowsum is applied to
            # the feat PSUM rows (which are q) during the PSUM->SBUF copy.
            for g in range(2):
                sd = tmp.tile([128, 128], F32, tag="sd", bufs=3)
                nc.scalar.activation(sd[:], den_ps[g], ACT.Sqrt)
                rd = tmp.tile([128, 128], F32, tag="rd", bufs=3)
                nc.vector.reciprocal(rd[:], sd[:])
                s = sm.tile([128, 128], F32, name=f"sc{g}", tag=f"sc{g}")
                nc.vector.tensor_mul(s[:], num_ps[g], rd[:])
                nc.vector.tensor_add(s[:], s[:], mi[:, 0:128])
                att = sm.tile([128, 128], F32, name=f"att{g}", tag=f"att{g}")
                nc.scalar.activation(att[:], s[:], ACT.Exp)
                rs = sm.tile([128, 1], F32, name=f"rs{g}", tag=f"rs{g}")
                nc.vector.reduce_sum(rs[:], att[:], axis=mybir.AxisListType.X)
                rrs = sm.tile([128, 1], F32, name=f"rrs{g}", tag=f"rrs{g}")
                nc.vector.reciprocal(rrs[:], rs[:])
                atp = psf.tile([128, 512], F32, tag="fps", bufs=2)
                nc.tensor.transpose(atp[:, 0:128], att[:], mi[:, 128:256])
                atb = sm.tile([128, 128], BF16, name=f"atb{g}", tag=f"atb{g}")
                nc.scalar.copy(atb[:], atp[:, 0:128])
                for c2 in range(2):
                    fsb = tmp.tile([128, 1024], BF16, tag="fsb", bufs=3)
                    for half in range(2):
                        c = 2 * c2 + half
                        cs = slice(512 * c, 512 * (c + 1))
                        fps = psf.tile([128, 512], F32, tag="fps", bufs=2)
                        nc.tensor.matmul(
                            fps[:], atb[:], hn[:, g, cs], start=True, stop=True
                        )
                        dst = fsb[:, 512 * half : 512 * (half + 1)]
                        if half == 0:
                            nc.scalar.activation(
                                dst, fps[:], ACT.Copy, scale=rrs[:, 0:1]
                            )
                        else:
                            nc.vector.tensor_scalar_mul(dst, fps[:], rrs[:, 0:1])
                    eng = nc.sync if c2 == 0 else nc.scalar
                    eng.dma_start(
                        feat_d[g, :, 1024 * c2 : 1024 * (c2 + 1)], fsb[:]
                    )

    _split_multi_waits(nc)
    return nc


# ---------------------------------------------------------------------------
# Host side
# ---------------------------------------------------------------------------

_PROG_CACHE = {}


def _get_prog(mode, zero_bias):
    key = (mode, zero_bias)
    if key not in _PROG_CACHE:
        _PROG_CACHE[key] = build_program(mode, zero_bias)
    return _PROG_CACHE[key]


def _xnp(mode):
    if mode == "fp8":
        return ml_dtypes.float8_e4m3
    return np.float32 if mode == "f32r" else ml_dtypes.bfloat16


def _prep_shared(W_hy, b_hy, W_hg, b_hg, W_qy, b_qy, W_qg, b_qg, W_att, mode):
    xnp = _xnp(mode)
    ws = WSCALE if mode == "fp8" else 1.0

    def reblock(W):
        # [IN, H] -> [128, MC, KC, 128]; Wr[p, m, k, h] = W[128k+p, 128m+h]
        return (W.reshape(KC, 128, MC, 128) * ws).transpose(1, 2, 0, 3).astype(xnp)

    def bvec(b):
        return np.ascontiguousarray(b.reshape(MC, 128).T).astype(np.float32)

    # causal 32x32 blocks on the diagonal, -1e30 everywhere else (kills
    # cross-example terms inside the 128-row group at exp time)
    i = np.arange(128)
    same_block = (i[:, None] // 32) == (i[None, :] // 32)
    causal = (i[None, :] % 32) <= (i[:, None] % 32)
    mask = np.where(same_block & causal, 0.0, NEG).astype(np.float32)
    # [MC, 128, 2, KC, 128]
    wh = np.ascontiguousarray(
        np.stack([reblock(W_hy), reblock(W_hg)], axis=2).transpose(1, 0, 2, 3, 4)
    )
    wq = np.ascontiguousarray(
        np.stack([reblock(W_qy), reblock(W_qg)], axis=2).transpose(1, 0, 2, 3, 4)
    )
    watt = bvec(W_att)
    if mode == "fp8":
        watt = watt / (ws * ws)
    bw = np.ascontiguousarray(
        np.stack([bvec(b_qy), bvec(b_qg), bvec(b_hy), bvec(b_hg), watt], axis=1)
    )
    mi = np.ascontiguousarray(
        np.concatenate([mask, np.eye(128, dtype=np.float32)], axis=1)
    )
    return {"wh": wh, "wq": wq, "bw": bw, "mi": mi}, xnp


def kernel(
    hist, ques, W_hy, b_hy, W_hg, b_hg, W_qy, b_qy, W_qg, b_qg, W_att, b_att,
    mode="fp8", trace=False,
):
    from concourse.bass_utils import run_bass_kernel_spmd

    hist = np.asarray(hist, np.float32)
    ques = np.asarray(ques, np.float32)
    nc = _get_prog(mode, True)
    shared, xnp = _prep_shared(
        np.asarray(W_hy, np.float32), np.asarray(b_hy, np.float32),
        np.asarray(W_hg, np.float32), np.asarray(b_hg, np.float32),
        np.asarray(W_qy, np.float32), np.asarray(b_qy, np.float32),
        np.asarray(W_qg, np.float32), np.asarray(b_qg, np.float32),
        np.asarray(W_att, np.float32), mode,
    )
    in_maps = []
    for c in range(NCORES):
        hs = hist[c * BL : (c + 1) * BL].reshape(BR, IN)
        qs = ques[c * BL : (c + 1) * BL].reshape(BR, IN)
        im = dict(shared)
        # [128, KC, BR]; qt[p, k, b] = qs[b, 128k+p]
        im["qt"] = np.ascontiguousarray(
            qs.T.reshape(KC, 128, BR).transpose(1, 0, 2)
        ).astype(xnp)
        im["ht"] = np.ascontiguousarray(
            hs.T.reshape(KC, 128, BR).transpose(1, 0, 2)
        ).astype(xnp)
        # [128, 2, IN]; hn[p, t, d] = hs[128t+p, d]
        im["hn"] = np.ascontiguousarray(
            hs.reshape(2, 128, IN).transpose(1, 0, 2)
        ).astype(ml_dtypes.bfloat16)
        in_maps.append(im)

    res = run_bass_kernel_spmd(
        nc, in_maps, core_ids=list(range(NCORES)), trace=trace
    )
    feat = np.concatenate(
        [
            r["feat"].astype(np.float32).reshape(BL, R, IN)
            for r in res.results
        ],
        axis=0,
    )
    if trace:
        return feat, res
    return feat


# revision 17
# speedup vs baseline: 1.0882x; 1.0882x over previous
"""Trainium2 Bass kernel for nn_H_ATT (GatedTrans pair-attention block).

Math (per example):
  HE = tanh(hist@W_hy+b_hy) * lrelu(hist@W_hg+b_hg)      [R, H]
  QE = tanh(ques@W_qy+b_qy) * lrelu(ques@W_qg+b_qg)      [R, H]
  num[q,h]  = sum_k QE[q,k]*W_att[k]*HE[h,k]
  den[q,h]  = sqrt(sum_k QE[q,k]^2 * HE[h,k]^2)
  s = num / max(den, eps)          (b_att cancels in softmax)
  att = causal_softmax(s)          (softmax*tril/renorm == masked softmax)
  feat = att @ hist                 [R, 2H]

Sharding: pure data parallel, 8 examples per core on 8 NeuronCores.

Perf structure (fp8 mode, default):
- All DRAM operand layouts are partition-major so every DMA reads long
  contiguous (4KB/partition) runs.
- The 4 big embedding GEMMs run as fp8e4 DoubleRow matmuls (2 k-tiles
  per instruction -> 2x PE throughput, 109ns/instr measured warm).
  Weights are pre-scaled by 64 on the host so W*64 ~ N(0,1.4) sits in
  e4m3's normal range; the scale is undone exactly: tanh gets
  scale=1/64, leaky_relu is positively homogeneous so the 64x rides
  through and cancels against watt/64^2 in num and scale=1/64 inside
  the squares for den.
- Weight-tile DMAs alternate between the two HWDGE rings (sync/SP and
  scalar/ACT) so descriptor generation pipelines; consts are merged
  into single transfers.
- The causal mask is -1e30 on ALL cross-example positions, so softmax
  is one full 128x128 exp + row reduce; the 1/rowsum is folded into
  the feat PSUM->SBUF copies (rows of feat PSUM are q).
- hist for the final feat matmul and the output are bf16.
"""

import numpy as np
import ml_dtypes

import bass_rust
import concourse.bass as bass
import concourse.mybir as mybir
import concourse.tile as tile
from concourse.vector_clock import ScopedClock

# ---------------------------------------------------------------------------
# Workaround: this walrus build accepts only ONE semaphore wait on an SP
# Drain, but TileContext's tail drain carries one wait per live semaphore.
# Split them across a chain of drains.
# ---------------------------------------------------------------------------


def _patched_drain_and_barrier(self, tick_clock, wait_clock):
    nc = self.nc
    drain_inst = nc.sync.drain()
    wait_clock.add_sem_waits(
        drain_inst.ins, ScopedClock({None: tick_clock.global_clock})
    )
    waits = list(drain_inst.ins.sync_info.on_wait)
    if len(waits) > 1:
        drain_inst.ins.sync_info = bass_rust.SyncInfo(
            on_wait=waits[:1], on_update=list(drain_inst.ins.sync_info.on_update)
        )
        for i in range(1, len(waits)):
            extra = nc.sync.drain()
            extra.ins.sync_info = bass_rust.SyncInfo(
                on_wait=waits[i : i + 1], on_update=[]
            )
    nc.all_engine_barrier()
    assert self.sems is not None
    popped = nc._tile_sem_poison_stack.pop()
    assert popped is self._sem_poison
    nc.clear_and_free_semaphores(list(self.sems.allocated().values()))
    nc.all_engine_barrier()


tile.TileContext._drain_and_barrier = _patched_drain_and_barrier


def _split_multi_waits(nc):
    """This walrus build accepts at most one semaphore wait per instruction.
    Hoist extra waits onto standalone EventSemaphore instructions inserted
    just before the owning instruction in the same engine's stream."""
    uid = [0]
    for f in nc.m.functions:
        for bb in f.blocks:
            out = []
            for inst in bb.instructions:
                si = inst.sync_info
                if si is not None and len(si.on_wait) > 1:
                    waits = list(si.on_wait)
                    for w in waits[:-1]:
                        nop = mybir.InstEventSemaphore(
                            name=f"I-waitsplit-{uid[0]}", ins=[], outs=[]
                        )
                        uid[0] += 1
                        nop.engine = inst.engine
                        nop.sync_info = bass_rust.SyncInfo(
                            on_wait=[w], on_update=[]
                        )
                        out.append(nop)
                    inst.sync_info = bass_rust.SyncInfo(
                        on_wait=[waits[-1]], on_update=list(si.on_update)
                    )
                out.append(inst)
            bb.instructions[:] = out

# ---------------------------------------------------------------------------

B, R, H, IN = 64, 32, 1024, 2048
NCORES = 8
BL = B // NCORES  # examples per core
BR = BL * R  # 256 rows per core
KC = IN // 128  # 16 contraction chunks
MC = H // 128  # 8 h chunks
NEG = -1.0e30
WSCALE = 64.0  # fp8 weight pre-scale (power of two)

F32 = mybir.dt.float32
BF16 = mybir.dt.bfloat16


def build_program(mode="fp8", zero_bias=True):
    """Build the per-core Bass program. mode selects the dtype of the
    big-GEMM operands (weights + transposed activations):
    fp8 (DoubleRow, weights pre-scaled), bf16, or f32r."""
    if mode == "fp8":
        xdt = mybir.dt.float8e4
        step = 2
        pmode = mybir.MatmulPerfMode.DoubleRow
        sinv = 1.0 / WSCALE
    else:
        xdt = mybir.dt.float32r if mode == "f32r" else BF16
        step = 1
        pmode = None
        sinv = 1.0

    nc = bass.Bass()
    qt_d = nc.dram_tensor("qt", [128, KC, BR], xdt, kind="ExternalInput")
    ht_d = nc.dram_tensor("ht", [128, KC, BR], xdt, kind="ExternalInput")
    hn_d = nc.dram_tensor("hn", [128, 2, IN], BF16, kind="ExternalInput")
    wh_d = nc.dram_tensor("wh", [MC, 128, 2, KC, 128], xdt, kind="ExternalInput")
    wq_d = nc.dram_tensor("wq", [MC, 128, 2, KC, 128], xdt, kind="ExternalInput")
    # [bqy, bqg, bhy, bhg, watt] stacked -> one DMA
    bw_d = nc.dram_tensor("bw", [128, 5, MC], F32, kind="ExternalInput")
    # [mask | ident] -> one DMA
    mi_d = nc.dram_tensor("mi", [128, 256], F32, kind="ExternalInput")
    feat_d = nc.dram_tensor("feat", [2, 128, IN], BF16, kind="ExternalOutput")

    ACT = mybir.ActivationFunctionType

    with tile.TileContext(nc) as tc:
        with (
            tc.tile_pool(name="sb", bufs=1) as big,
            tc.tile_pool(name="ps", bufs=1, space="PSUM") as psp,
        ):
            sm = big
            wts = big
            tmp = big
            pse = psp
            psnd = psp
            psf = psp
            # consts: one small transfer on the scalar ring; the sync ring
            # carries the whole weight stream
            bw = sm.tile([128, 5, MC], F32, tag="bw")
            nc.scalar.dma_start(bw[:], bw_d[:])
            # ques-transposed activations: first compute dependency; split
            # across the two rings so the first matmul starts sooner
            qt = big.tile([128, KC, BR], xdt, tag="qt")
            nc.sync.dma_start(qt[:, 0:8, :], qt_d[:, 0:8, :])
            nc.scalar.dma_start(qt[:, 8:16, :], qt_d[:, 8:16, :])

            # dummy activations with no data deps: the scalar engine runs
            # them during the initial DMA wait, so the Tanh/Lrelu/Square
            # PWP table loads happen off the critical path
            w0 = sm.tile([128, 1], F32, tag="w0")
            nc.scalar.activation(w0[:], bw[:, 0, 0:1], ACT.Tanh)
            nc.scalar.activation(w0[:], bw[:, 0, 0:1], ACT.Lrelu, alpha=0.01)
            nc.scalar.activation(w0[:], bw[:, 0, 0:1], ACT.Square)

            EDT = BF16
            he = big.tile([128, MC, BR], EDT, tag="he")
            he2 = big.tile([128, MC, BR], EDT, tag="he2")
            qew = big.tile([128, MC, BR], EDT, tag="qew")
            qe2 = big.tile([128, MC, BR], EDT, tag="qe2")

            num_ps = [
                psnd.tile([128, 128], F32, name=f"num{g}", tag=f"num{g}")[:]
                for g in range(2)
            ]
            den_ps = [
                psnd.tile([128, 128], F32, name=f"den{g}", tag=f"den{g}")[:]
                for g in range(2)
            ]

            def gated(xt, w_dram, iy, ig, m, split=False):
                """One contiguous y+g weight DMA (sync ring); big GEMM pair.
                Returns (ty, tg) [128, BR]: ty = tanh branch, tg = the
                (64x-scaled in fp8 mode) leaky_relu branch."""
                wt = wts.tile([128, 2, KC, 128], xdt, tag="wt", bufs=8)
                if split:
                    # y half first so the first matmuls start half a
                    # transfer earlier
                    nc.sync.dma_start(wt[:, 0], w_dram[m, :, 0])
                    nc.sync.dma_start(wt[:, 1], w_dram[m, :, 1])
                else:
                    nc.sync.dma_start(wt[:], w_dram[m])
                ps = pse.tile([128, 2 * BR], F32, tag="ps", bufs=2)
                psy, psg = ps[:, 0:BR], ps[:, BR : 2 * BR]
                for k in range(0, KC, step):
                    nc.tensor.matmul(
                        psy,
                        wt[:, 0, k : k + step, :] if step == 2 else wt[:, 0, k, :],
                        xt[:, k : k + step, :] if step == 2 else xt[:, k, :],
                        start=(k == 0), stop=(k + step == KC),
                        perf_mode=pmode,
                    )
                for k in range(0, KC, step):
                    nc.tensor.matmul(
                        psg,
                        wt[:, 1, k : k + step, :] if step == 2 else wt[:, 1, k, :],
                        xt[:, k : k + step, :] if step == 2 else xt[:, k, :],
                        start=(k == 0), stop=(k + step == KC),
                        perf_mode=pmode,
                    )
                ty = tmp.tile([128, BR], F32, tag="ty", bufs=3)
                nc.scalar.activation(
                    ty[:], psy, ACT.Tanh, bias=bw[:, iy, m : m + 1], scale=sinv
                )
                # leaky_relu(s*x) = s*leaky_relu(x): the 64x rides along
                tg = tmp.tile([128, BR], F32, tag="tg", bufs=3)
                nc.scalar.activation(
                    tg[:], psg, ACT.Lrelu, bias=bw[:, ig, m : m + 1], alpha=0.01
                )
                return ty, tg

            # ques embeddings (first: only needs qt + wq)
            for m in range(MC):
                ty, tg = gated(qt, wq_d, 0, 1, m, split=(m == 0))
                # qew = ty * (watt/64^2) * tg_scaled  -> qew_true/64
                nc.vector.scalar_tensor_tensor(
                    qew[:, m, :], ty[:], bw[:, 4, m : m + 1], tg[:],
                    op0=mybir.AluOpType.mult, op1=mybir.AluOpType.mult,
                )
                qe = tmp.tile([128, BR], F32, tag="qe", bufs=3)
                nc.vector.tensor_mul(qe[:], ty[:], tg[:])
                # (qe_scaled/64)^2 = qe_true^2
                nc.scalar.activation(qe2[:, m, :], qe[:], ACT.Square, scale=sinv)
                if m == 2:
                    # hist inputs on the scalar ring once the early rush
                    # (qt + first weight tiles) has drained
                    ht = big.tile([128, KC, BR], xdt, tag="ht")
                    nc.scalar.dma_start(ht[:], ht_d[:])
                if m == 5:
                    hn = big.tile([128, 2, IN], BF16, tag="hn")
                    nc.scalar.dma_start(hn[:], hn_d[:])
                if m == 6:
                    mi = sm.tile([128, 256], F32, tag="mi")
                    nc.scalar.dma_start(mi[:], mi_d[:])


            # hist embeddings + num/den accumulation per chunk
            for m in range(MC):
                ty, tg = gated(ht, wh_d, 2, 3, m)
                nc.vector.tensor_mul(he[:, m, :], ty[:], tg[:])
                for g in range(2):
                    sl = slice(128 * g, 128 * (g + 1))
                    nc.tensor.matmul(
                        num_ps[g], qew[:, m, sl], he[:, m, sl],
                        start=(m == 0), stop=(m == MC - 1),
                    )
                nc.scalar.activation(he2[:, m, :], he[:, m, :], ACT.Square, scale=sinv)
                for g in range(2):
                    sl = slice(128 * g, 128 * (g + 1))
                    nc.tensor.matmul(
                        den_ps[g], qe2[:, m, sl], he2[:, m, sl],
                        start=(m == 0), stop=(m == MC - 1),
                    )

            # Dummy Ln/Exp anchored on the last hist chunk's output: they
            # run right after the m=7 scalar ops, so the ~1.3us PWP table
            # loads overlap the trailing num/den matmuls instead of the
            # serial tail path below. (No-dep dummies get hoisted by the
            # Tile scheduler into the middle of the embedding phase and
    # thrash the Tanh/Lrelu/Square tables -- measured, do not.)
            warm = sm.tile([128, 1], F32, tag="warm")
            nc.scalar.activation(warm[:], he2[:, MC - 1, 0:1], ACT.Ln)
            nc.scalar.activation(warm[:], warm[:], ACT.Exp)

            # attention tail + feat: mask is -1e30 off the causal diagonal
            # blocks, so exp of the full tile zeroes cross-example terms and
            # the row sum is the softmax denominator; 1/rowsum is applied to
            # the feat PSUM rows (which are q) during the PSUM->SBUF copy.
            for g in range(2):
                # 1/sqrt(den2) = exp(-0.5*ln(den2)): two table ops on the
                # scalar engine; avoids the 950ns DVE reciprocal
                sd = tmp.tile([128, 128], F32, tag="sd", bufs=3)
                nc.scalar.activation(sd[:], den_ps[g], ACT.Ln)
                rd = tmp.tile([128, 128], F32, tag="rd", bufs=3)
                nc.scalar.activation(rd[:], sd[:], ACT.Exp, scale=-0.5)
                s = sm.tile([128, 128], F32, name=f"sc{g}", tag=f"sc{g}")
                nc.vector.tensor_mul(s[:], num_ps[g], rd[:])
                nc.vector.tensor_add(s[:], s[:], mi[:, 0:128])
                att = sm.tile([128, 128], F32, name=f"att{g}", tag=f"att{g}")
                nc.scalar.activation(att[:], s[:], ACT.Exp)
                rs = sm.tile([128, 1], F32, name=f"rs{g}", tag=f"rs{g}")
                nc.vector.reduce_sum(rs[:], att[:], axis=mybir.AxisListType.X)
                lrs = sm.tile([128, 1], F32, name=f"lrs{g}", tag=f"lrs{g}")
                nc.scalar.activation(lrs[:], rs[:], ACT.Ln)
                rrs = sm.tile([128, 1], F32, name=f"rrs{g}", tag=f"rrs{g}")
                nc.scalar.activation(rrs[:], lrs[:], ACT.Exp, scale=-1.0)
                atp = psf.tile([128, 512], F32, tag="fps", bufs=2)
                nc.tensor.transpose(atp[:, 0:128], att[:], mi[:, 128:256])
                atb = sm.tile([128, 128], BF16, name=f"atb{g}", tag=f"atb{g}")
                nc.scalar.copy(atb[:], atp[:, 0:128])
                for c2 in range(2):
                    fsb = tmp.tile([128, 1024], BF16, tag="fsb", bufs=3)
                    for half in range(2):
                        c = 2 * c2 + half
                        cs = slice(512 * c, 512 * (c + 1))
                        fps = psf.tile([128, 512], F32, tag="fps", bufs=2)
                        nc.tensor.matmul(
                            fps[:], atb[:], hn[:, g, cs], start=True, stop=True
                        )
                        dst = fsb[:, 512 * half : 512 * (half + 1)]
                        if half == 0:
                            nc.scalar.activation(
                                dst, fps[:], ACT.Copy, scale=rrs[:, 0:1]
                            )
                        else:
                            nc.vector.tensor_scalar_mul(dst, fps[:], rrs[:, 0:1])
                    eng = nc.sync if c2 == 0 else nc.scalar
                    eng.dma_start(
                        feat_d[g, :, 1024 * c2 : 1024 * (c2 + 1)], fsb[:]
                    )

    _split_multi_waits(nc)
    return nc


# ---------------------------------------------------------------------------
# Host side
# ---------------------------------------------------------------------------

_PROG_CACHE = {}


def _get_prog(mode, zero_bias):
    key = (mode, zero_bias)
    if key not in _PROG_CACHE:
        _PROG_CACHE[key] = build_program(mode, zero_bias)
    return _PROG_CACHE[key]


def _xnp(mode):
    if mode == "fp8":
        return ml_dtypes.float8_e4m3
    return np.float32 if mode == "f32r" else ml_dtypes.bfloat16


def _prep_shared(W_hy, b_hy, W_hg, b_hg, W_qy, b_qy, W_qg, b_qg, W_att, mode):
    xnp = _xnp(mode)
    ws = WSCALE if mode == "fp8" else 1.0

    def reblock(W):
        # [IN, H] -> [128, MC, KC, 128]; Wr[p, m, k, h] = W[128k+p, 128m+h]
        return (W.reshape(KC, 128, MC, 128) * ws).transpose(1, 2, 0, 3).astype(xnp)

    def bvec(b):
        return np.ascontiguousarray(b.reshape(MC, 128).T).astype(np.float32)

    # causal 32x32 blocks on the diagonal, -1e30 everywhere else (kills
    # cross-example terms inside the 128-row group at exp time)
    i = np.arange(128)
    same_block = (i[:, None] // 32) == (i[None, :] // 32)
    causal = (i[None, :] % 32) <= (i[:, None] % 32)
    mask = np.where(same_block & causal, 0.0, NEG).astype(np.float32)
    # [MC, 128, 2, KC, 128]
    wh = np.ascontiguousarray(
        np.stack([reblock(W_hy), reblock(W_hg)], axis=2).transpose(1, 0, 2, 3, 4)
    )
    wq = np.ascontiguousarray(
        np.stack([reblock(W_qy), reblock(W_qg)], axis=2).transpose(1, 0, 2, 3, 4)
    )
    watt = bvec(W_att)
    if mode == "fp8":
        watt = watt / (ws * ws)
    bw = np.ascontiguousarray(
        np.stack([bvec(b_qy), bvec(b_qg), bvec(b_hy), bvec(b_hg), watt], axis=1)
    )
    mi = np.ascontiguousarray(
        np.concatenate([mask, np.eye(128, dtype=np.float32)], axis=1)
    )
    return {"wh": wh, "wq": wq, "bw": bw, "mi": mi}, xnp


def kernel(
    hist, ques, W_hy, b_hy, W_hg, b_hg, W_qy, b_qy, W_qg, b_qg, W_att, b_att,
    mode="fp8", trace=False,
):
    from concourse.bass_utils import run_bass_kernel_spmd

    hist = np.asarray(hist, np.float32)
    ques = np.asarray(ques, np.float32)
    nc = _get_prog(mode, True)
    shared, xnp = _prep_shared(
        np.asarray(W_hy, np.float32), np.asarray(b_hy, np.float32),
        np.asarray(W_hg, np.float32), np.asarray(b_hg, np.float32),
        np.asarray(W_qy, np.float32), np.asarray(b_qy, np.float32),
        np.asarray(W_qg, np.float32), np.asarray(b_qg, np.float32),
        np.asarray(W_att, np.float32), mode,
    )
    in_maps = []
    for c in range(NCORES):
        hs = hist[c * BL : (c + 1) * BL].reshape(BR, IN)
        qs = ques[c * BL : (c + 1) * BL].reshape(BR, IN)
        im = dict(shared)
        # [128, KC, BR]; qt[p, k, b] = qs[b, 128k+p]
        im["qt"] = np.ascontiguousarray(
            qs.T.reshape(KC, 128, BR).transpose(1, 0, 2)
        ).astype(xnp)
        im["ht"] = np.ascontiguousarray(
            hs.T.reshape(KC, 128, BR).transpose(1, 0, 2)
        ).astype(xnp)
        # [128, 2, IN]; hn[p, t, d] = hs[128t+p, d]
        im["hn"] = np.ascontiguousarray(
            hs.reshape(2, 128, IN).transpose(1, 0, 2)
        ).astype(ml_dtypes.bfloat16)
        in_maps.append(im)

    res = run_bass_kernel_spmd(
        nc, in_maps, core_ids=list(range(NCORES)), trace=trace
    )
    feat = np.concatenate(
        [
            r["feat"].astype(np.float32).reshape(BL, R, IN)
            for r in res.results
        ],
        axis=0,
    )
    if trace:
        return feat, res
    return feat


# revision 19
# speedup vs baseline: 1.1280x; 1.0366x over previous
"""Trainium2 Bass kernel for nn_H_ATT (GatedTrans pair-attention block).

Math (per example):
  HE = tanh(hist@W_hy+b_hy) * lrelu(hist@W_hg+b_hg)      [R, H]
  QE = tanh(ques@W_qy+b_qy) * lrelu(ques@W_qg+b_qg)      [R, H]
  num[q,h]  = sum_k QE[q,k]*W_att[k]*HE[h,k]
  den[q,h]  = sqrt(sum_k QE[q,k]^2 * HE[h,k]^2)
  s = num / max(den, eps)          (b_att cancels in softmax)
  att = causal_softmax(s)          (softmax*tril/renorm == masked softmax)
  feat = att @ hist                 [R, 2H]

Sharding: pure data parallel, 8 examples per core on 8 NeuronCores.

Perf structure (fp8 mode, default):
- All DRAM operand layouts are partition-major so every DMA reads long
  contiguous (4KB/partition) runs.
- The 4 big embedding GEMMs run as fp8e4 DoubleRow matmuls (2 k-tiles
  per instruction -> 2x PE throughput, 109ns/instr measured warm).
  Weights are pre-scaled by 64 on the host so W*64 ~ N(0,1.4) sits in
  e4m3's normal range; the scale is undone exactly: tanh gets
  scale=1/64, leaky_relu is positively homogeneous so the 64x rides
  through and cancels against watt/64^2 in num and scale=1/64 inside
  the squares for den.
- Weight-tile DMAs alternate between the two HWDGE rings (sync/SP and
  scalar/ACT) so descriptor generation pipelines; consts are merged
  into single transfers.
- The causal mask is -1e30 on ALL cross-example positions, so softmax
  is one full 128x128 exp + row reduce; the 1/rowsum is folded into
  the feat PSUM->SBUF copies (rows of feat PSUM are q).
- hist for the final feat matmul and the output are bf16.
"""

import numpy as np
import ml_dtypes

import bass_rust
import concourse.bass as bass
import concourse.mybir as mybir
import concourse.tile as tile
from concourse.vector_clock import ScopedClock

# ---------------------------------------------------------------------------
# Workaround: this walrus build accepts only ONE semaphore wait on an SP
# Drain, but TileContext's tail drain carries one wait per live semaphore.
# Split them across a chain of drains.
# ---------------------------------------------------------------------------


def _patched_drain_and_barrier(self, tick_clock, wait_clock):
    nc = self.nc
    drain_inst = nc.sync.drain()
    wait_clock.add_sem_waits(
        drain_inst.ins, ScopedClock({None: tick_clock.global_clock})
    )
    waits = list(drain_inst.ins.sync_info.on_wait)
    if len(waits) > 1:
        drain_inst.ins.sync_info = bass_rust.SyncInfo(
            on_wait=waits[:1], on_update=list(drain_inst.ins.sync_info.on_update)
        )
        for i in range(1, len(waits)):
            extra = nc.sync.drain()
            extra.ins.sync_info = bass_rust.SyncInfo(
                on_wait=waits[i : i + 1], on_update=[]
            )
    nc.all_engine_barrier()
    assert self.sems is not None
    popped = nc._tile_sem_poison_stack.pop()
    assert popped is self._sem_poison
    nc.clear_and_free_semaphores(list(self.sems.allocated().values()))
    nc.all_engine_barrier()


tile.TileContext._drain_and_barrier = _patched_drain_and_barrier


def _split_multi_waits(nc):
    """This walrus build accepts at most one semaphore wait per instruction.
    Hoist extra waits onto standalone EventSemaphore instructions inserted
    just before the owning instruction in the same engine's stream."""
    uid = [0]
    for f in nc.m.functions:
        for bb in f.blocks:
            out = []
            for inst in bb.instructions:
                si = inst.sync_info
                if si is not None and len(si.on_wait) > 1:
                    waits = list(si.on_wait)
                    for w in waits[:-1]:
                        nop = mybir.InstEventSemaphore(
                            name=f"I-waitsplit-{uid[0]}", ins=[], outs=[]
                        )
                        uid[0] += 1
                        nop.engine = inst.engine
                        nop.sync_info = bass_rust.SyncInfo(
                            on_wait=[w], on_update=[]
                        )
                        out.append(nop)
                    inst.sync_info = bass_rust.SyncInfo(
                        on_wait=[waits[-1]], on_update=list(si.on_update)
                    )
                out.append(inst)
            bb.instructions[:] = out

# ---------------------------------------------------------------------------

B, R, H, IN = 64, 32, 1024, 2048
NCORES = 8
BL = B // NCORES  # examples per core
BR = BL * R  # 256 rows per core
KC = IN // 128  # 16 contraction chunks
MC = H // 128  # 8 h chunks
NEG = -1.0e30
WSCALE = 64.0  # fp8 weight pre-scale (power of two)

F32 = mybir.dt.float32
BF16 = mybir.dt.bfloat16


def build_program(mode="fp8", zero_bias=True):
    """Build the per-core Bass program. mode selects the dtype of the
    big-GEMM operands (weights + transposed activations):
    fp8 (DoubleRow, weights pre-scaled), bf16, or f32r."""
    if mode == "fp8":
        xdt = mybir.dt.float8e4
        step = 2
        pmode = mybir.MatmulPerfMode.DoubleRow
        sinv = 1.0 / WSCALE
    else:
        xdt = mybir.dt.float32r if mode == "f32r" else BF16
        step = 1
        pmode = None
        sinv = 1.0

    nc = bass.Bass()
    qt_d = nc.dram_tensor("qt", [128, KC, BR], xdt, kind="ExternalInput")
    ht_d = nc.dram_tensor("ht", [128, KC, BR], xdt, kind="ExternalInput")
    hn_d = nc.dram_tensor("hn", [128, 2, IN], BF16, kind="ExternalInput")
    wh_d = nc.dram_tensor("wh", [MC, 128, 2, KC, 128], xdt, kind="ExternalInput")
    wq_d = nc.dram_tensor("wq", [MC, 128, 2, KC, 128], xdt, kind="ExternalInput")
    # [bqy, bqg, bhy, bhg, watt] stacked -> one DMA
    bw_d = nc.dram_tensor("bw", [128, 5, MC], F32, kind="ExternalInput")
    # [mask | ident] -> one DMA
    mi_d = nc.dram_tensor("mi", [128, 256], F32, kind="ExternalInput")
    feat_d = nc.dram_tensor("feat", [2, 128, IN], BF16, kind="ExternalOutput")

    ACT = mybir.ActivationFunctionType

    with tile.TileContext(nc) as tc:
        with (
            tc.tile_pool(name="sb", bufs=1) as big,
            tc.tile_pool(name="ps", bufs=1, space="PSUM") as psp,
        ):
            sm = big
            wts = big
            tmp = big
            pse = psp
            psnd = psp
            psf = psp
            # consts: one small transfer on the scalar ring; the sync ring
            # carries the whole weight stream
            bw = sm.tile([128, 5, MC], F32, tag="bw")
            nc.scalar.dma_start(bw[:], bw_d[:])
            # ques-transposed activations: first compute dependency; split
            # across the two rings so the first matmul starts sooner
            qt = big.tile([128, KC, BR], xdt, tag="qt")
            nc.sync.dma_start(qt[:, 0:8, :], qt_d[:, 0:8, :])
            nc.scalar.dma_start(qt[:, 8:16, :], qt_d[:, 8:16, :])

            # dummy activations with no data deps: the scalar engine runs
            # them during the initial DMA wait, so the Tanh/Lrelu/Square
            # PWP table loads happen off the critical path
            ones = sm.tile([128, 1], BF16, tag="ones")
            nc.vector.memset(ones[:], 1.0)
            w0 = sm.tile([128, 1], F32, tag="w0")
            nc.scalar.activation(w0[:], bw[:, 0, 0:1], ACT.Tanh)
            nc.scalar.activation(w0[:], bw[:, 0, 0:1], ACT.Lrelu, alpha=0.01)
            nc.scalar.activation(w0[:], bw[:, 0, 0:1], ACT.Square)

            EDT = BF16
            he = big.tile([128, MC, BR], EDT, tag="he")
            he2 = big.tile([128, MC, BR], EDT, tag="he2")
            qew = big.tile([128, MC, BR], EDT, tag="qew")
            qe2 = big.tile([128, MC, BR], EDT, tag="qe2")

            num_ps = [
                psnd.tile([128, 128], F32, name=f"num{g}", tag=f"num{g}")[:]
                for g in range(2)
            ]
            den_ps = [
                psnd.tile([128, 128], F32, name=f"den{g}", tag=f"den{g}")[:]
                for g in range(2)
            ]

            def gated(xt, w_dram, iy, ig, m, split=False):
                """One contiguous y+g weight DMA (sync ring); big GEMM pair.
                Returns (ty, tg) [128, BR]: ty = tanh branch, tg = the
                (64x-scaled in fp8 mode) leaky_relu branch."""
                wt = wts.tile([128, 2, KC, 128], xdt, tag="wt", bufs=8)
                if split:
                    # y half first so the first matmuls start half a
                    # transfer earlier
                    nc.sync.dma_start(wt[:, 0], w_dram[m, :, 0])
                    nc.sync.dma_start(wt[:, 1], w_dram[m, :, 1])
                else:
                    nc.sync.dma_start(wt[:], w_dram[m])
                ps = pse.tile([128, 2 * BR], F32, tag="ps", bufs=2)
                psy, psg = ps[:, 0:BR], ps[:, BR : 2 * BR]
                for k in range(0, KC, step):
                    nc.tensor.matmul(
                        psy,
                        wt[:, 0, k : k + step, :] if step == 2 else wt[:, 0, k, :],
                        xt[:, k : k + step, :] if step == 2 else xt[:, k, :],
                        start=(k == 0), stop=(k + step == KC),
                        perf_mode=pmode,
                    )
                for k in range(0, KC, step):
                    nc.tensor.matmul(
                        psg,
                        wt[:, 1, k : k + step, :] if step == 2 else wt[:, 1, k, :],
                        xt[:, k : k + step, :] if step == 2 else xt[:, k, :],
                        start=(k == 0), stop=(k + step == KC),
                        perf_mode=pmode,
                    )
                ty = tmp.tile([128, BR], F32, tag="ty", bufs=3)
                nc.scalar.activation(
                    ty[:], psy, ACT.Tanh, bias=bw[:, iy, m : m + 1], scale=sinv
                )
                # leaky_relu(s*x) = s*leaky_relu(x): the 64x rides along
                tg = tmp.tile([128, BR], F32, tag="tg", bufs=3)
                nc.scalar.activation(
                    tg[:], psg, ACT.Lrelu, bias=bw[:, ig, m : m + 1], alpha=0.01
                )
                return ty, tg

            # ques embeddings (first: only needs qt + wq)
            for m in range(MC):
                ty, tg = gated(qt, wq_d, 0, 1, m, split=(m == 0))
                # qew = ty * (watt/64^2) * tg_scaled  -> qew_true/64
                nc.vector.scalar_tensor_tensor(
                    qew[:, m, :], ty[:], bw[:, 4, m : m + 1], tg[:],
                    op0=mybir.AluOpType.mult, op1=mybir.AluOpType.mult,
                )
                qe = tmp.tile([128, BR], F32, tag="qe", bufs=3)
                nc.vector.tensor_mul(qe[:], ty[:], tg[:])
                # (qe_scaled/64)^2 = qe_true^2
                nc.scalar.activation(qe2[:, m, :], qe[:], ACT.Square, scale=sinv)
                if m == 4:
                    # hist inputs on the scalar ring once the early rush
                    # (qt + first weight tiles) has drained
                    ht = big.tile([128, KC, BR], xdt, tag="ht")
                    nc.scalar.dma_start(ht[:], ht_d[:])
                if m == 6:
                    hn = big.tile([128, 2, IN], BF16, tag="hn")
                    nc.scalar.dma_start(hn[:], hn_d[:])
                if m == 7:
                    mi = sm.tile([128, 256], F32, tag="mi")
                    nc.scalar.dma_start(mi[:], mi_d[:])


            # hist embeddings + num/den accumulation per chunk
            for m in range(MC):
                ty, tg = gated(ht, wh_d, 2, 3, m)
                nc.vector.tensor_mul(he[:, m, :], ty[:], tg[:])
                for g in range(2):
                    sl = slice(128 * g, 128 * (g + 1))
                    nc.tensor.matmul(
                        num_ps[g], qew[:, m, sl], he[:, m, sl],
                        start=(m == 0), stop=(m == MC - 1),
                    )
                nc.scalar.activation(he2[:, m, :], he[:, m, :], ACT.Square, scale=sinv)
                for g in range(2):
                    sl = slice(128 * g, 128 * (g + 1))
                    nc.tensor.matmul(
                        den_ps[g], qe2[:, m, sl], he2[:, m, sl],
                        start=(m == 0), stop=(m == MC - 1),
                    )

            # Dummy Ln/Exp anchored on the last hist chunk's output: they
            # run right after the m=7 scalar ops, so the ~1.3us PWP table
            # loads overlap the trailing num/den matmuls instead of the
            # serial tail path below. (No-dep dummies get hoisted by the
            # Tile scheduler into the middle of the embedding phase and
    # thrash the Tanh/Lrelu/Square tables -- measured, do not.)
            warm = sm.tile([128, 1], F32, tag="warm")
            nc.scalar.activation(warm[:], he2[:, MC - 1, 0:1], ACT.Ln)
            nc.scalar.activation(warm[:], warm[:], ACT.Exp)

            # attention tail + feat: mask is -1e30 off the causal diagonal
            # blocks, so exp of the full tile zeroes cross-example terms and
            # the row sum is the softmax denominator; 1/rowsum is applied to
            # the feat PSUM rows (which are q) during the PSUM->SBUF copy.
            for g in range(2):
                # 1/sqrt(den2) = exp(-0.5*ln(den2)): two table ops on the
                # scalar engine; avoids the 950ns DVE reciprocal
                sd = tmp.tile([128, 128], F32, tag="sd", bufs=3)
                nc.scalar.activation(sd[:], den_ps[g], ACT.Ln)
                rd = tmp.tile([128, 128], F32, tag="rd", bufs=3)
                nc.scalar.activation(rd[:], sd[:], ACT.Exp, scale=-0.5)
                s = sm.tile([128, 128], F32, name=f"sc{g}", tag=f"sc{g}")
                nc.vector.tensor_mul(s[:], num_ps[g], rd[:])
                nc.vector.tensor_add(s[:], s[:], mi[:, 0:128])
                # exp commutes with transpose: transpose the scores (PE waits
                # only ~1.3us after the last num/den matmul, keeping HAM at
                # full clock), then exp PSUM->SBUF lands att^T in bf16
                # directly -- no separate att tile or PSUM copy.
                atp = psf.tile([128, 512], F32, tag="fps", bufs=2)
                nc.tensor.transpose(atp[:, 0:128], s[:], mi[:, 128:256])
                atb = sm.tile([128, 128], BF16, name=f"atb{g}", tag=f"atb{g}")
                nc.scalar.activation(atb[:], atp[:, 0:128], ACT.Exp)
                # softmax row sums: ones-matmul over the partition dim of
                # att^T (DVE cannot reduce across partitions)
                rsp = psf.tile([128, 512], F32, tag="fps", bufs=2)
                nc.tensor.matmul(
                    rsp[:, 0:1], atb[:], ones[:], start=True, stop=True
                )
                lrs = sm.tile([128, 1], F32, name=f"lrs{g}", tag=f"lrs{g}")
                nc.scalar.activation(lrs[:], rsp[:, 0:1], ACT.Ln)
                rrs = sm.tile([128, 1], F32, name=f"rrs{g}", tag=f"rrs{g}")
                nc.scalar.activation(rrs[:], lrs[:], ACT.Exp, scale=-1.0)
                for c2 in range(2):
                    fsb = tmp.tile([128, 1024], BF16, tag="fsb", bufs=3)
                    for half in range(2):
                        c = 2 * c2 + half
                        cs = slice(512 * c, 512 * (c + 1))
                        fps = psf.tile([128, 512], F32, tag="fps", bufs=2)
                        nc.tensor.matmul(
                            fps[:], atb[:], hn[:, g, cs], start=True, stop=True
                        )
                        dst = fsb[:, 512 * half : 512 * (half + 1)]
                        if half == 0:
                            nc.scalar.activation(
                                dst, fps[:], ACT.Copy, scale=rrs[:, 0:1]
                            )
                        else:
                            nc.vector.tensor_scalar_mul(dst, fps[:], rrs[:, 0:1])
                    eng = nc.sync if c2 == 0 else nc.scalar
                    eng.dma_start(
                        feat_d[g, :, 1024 * c2 : 1024 * (c2 + 1)], fsb[:]
                    )

    _split_multi_waits(nc)
    return nc


# ---------------------------------------------------------------------------
# Host side
# ---------------------------------------------------------------------------

_PROG_CACHE = {}


def _get_prog(mode, zero_bias):
    key = (mode, zero_bias)
    if key not in _PROG_CACHE:
        _PROG_CACHE[key] = build_program(mode, zero_bias)
    return _PROG_CACHE[key]


def _xnp(mode):
    if mode == "fp8":
        return ml_dtypes.float8_e4m3
    return np.float32 if mode == "f32r" else ml_dtypes.bfloat16


def _prep_shared(W_hy, b_hy, W_hg, b_hg, W_qy, b_qy, W_qg, b_qg, W_att, mode):
    xnp = _xnp(mode)
    ws = WSCALE if mode == "fp8" else 1.0

    def reblock(W):
        # [IN, H] -> [128, MC, KC, 128]; Wr[p, m, k, h] = W[128k+p, 128m+h]
        return (W.reshape(KC, 128, MC, 128) * ws).transpose(1, 2, 0, 3).astype(xnp)

    def bvec(b):
        return np.ascontiguousarray(b.reshape(MC, 128).T).astype(np.float32)

    # causal 32x32 blocks on the diagonal, -1e30 everywhere else (kills
    # cross-example terms inside the 128-row group at exp time)
    i = np.arange(128)
    same_block = (i[:, None] // 32) == (i[None, :] // 32)
    causal = (i[None, :] % 32) <= (i[:, None] % 32)
    mask = np.where(same_block & causal, 0.0, NEG).astype(np.float32)
    # [MC, 128, 2, KC, 128]
    wh = np.ascontiguousarray(
        np.stack([reblock(W_hy), reblock(W_hg)], axis=2).transpose(1, 0, 2, 3, 4)
    )
    wq = np.ascontiguousarray(
        np.stack([reblock(W_qy), reblock(W_qg)], axis=2).transpose(1, 0, 2, 3, 4)
    )
    watt = bvec(W_att)
    if mode == "fp8":
        watt = watt / (ws * ws)
    bw = np.ascontiguousarray(
        np.stack([bvec(b_qy), bvec(b_qg), bvec(b_hy), bvec(b_hg), watt], axis=1)
    )
    mi = np.ascontiguousarray(
        np.concatenate([mask, np.eye(128, dtype=np.float32)], axis=1)
    )
    return {"wh": wh, "wq": wq, "bw": bw, "mi": mi}, xnp


def kernel(
    hist, ques, W_hy, b_hy, W_hg, b_hg, W_qy, b_qy, W_qg, b_qg, W_att, b_att,
    mode="fp8", trace=False,
):
    from concourse.bass_utils import run_bass_kernel_spmd

    hist = np.asarray(hist, np.float32)
    ques = np.asarray(ques, np.float32)
    nc = _get_prog(mode, True)
    shared, xnp = _prep_shared(
        np.asarray(W_hy, np.float32), np.asarray(b_hy, np.float32),
        np.asarray(W_hg, np.float32), np.asarray(b_hg, np.float32),
        np.asarray(W_qy, np.float32), np.asarray(b_qy, np.float32),
        np.asarray(W_qg, np.float32), np.asarray(b_qg, np.float32),
        np.asarray(W_att, np.float32), mode,
    )
    in_maps = []
    for c in range(NCORES):
        hs = hist[c * BL : (c + 1) * BL].reshape(BR, IN)
        qs = ques[c * BL : (c + 1) * BL].reshape(BR, IN)
        im = dict(shared)
        # [128, KC, BR]; qt[p, k, b] = qs[b, 128k+p]
        im["qt"] = np.ascontiguousarray(
            qs.T.reshape(KC, 128, BR).transpose(1, 0, 2)
        ).astype(xnp)
        im["ht"] = np.ascontiguousarray(
            hs.T.reshape(KC, 128, BR).transpose(1, 0, 2)
        ).astype(xnp)
        # [128, 2, IN]; hn[p, t, d] = hs[128t+p, d]
        im["hn"] = np.ascontiguousarray(
            hs.reshape(2, 128, IN).transpose(1, 0, 2)
        ).astype(ml_dtypes.bfloat16)
        in_maps.append(im)

    res = run_bass_kernel_spmd(
        nc, in_maps, core_ids=list(range(NCORES)), trace=trace
    )
    feat = np.concatenate(
        [
            r["feat"].astype(np.float32).reshape(BL, R, IN)
            for r in res.results
        ],
        axis=0,
    )
    if trace:
        return feat, res
    return feat


# revision 20
# speedup vs baseline: 1.1540x; 1.0230x over previous
"""Trainium2 Bass kernel for nn_H_ATT (GatedTrans pair-attention block).

Math (per example):
  HE = tanh(hist@W_hy+b_hy) * lrelu(hist@W_hg+b_hg)      [R, H]
  QE = tanh(ques@W_qy+b_qy) * lrelu(ques@W_qg+b_qg)      [R, H]
  num[q,h]  = sum_k QE[q,k]*W_att[k]*HE[h,k]
  den[q,h]  = sqrt(sum_k QE[q,k]^2 * HE[h,k]^2)
  s = num / max(den, eps)          (b_att cancels in softmax)
  att = causal_softmax(s)          (softmax*tril/renorm == masked softmax)
  feat = att @ hist                 [R, 2H]

Sharding: pure data parallel, 8 examples per core on 8 NeuronCores.

Perf structure (fp8 mode, default):
- All DRAM operand layouts are partition-major so every DMA reads long
  contiguous (4KB/partition) runs.
- The 4 big embedding GEMMs run as fp8e4 DoubleRow matmuls (2 k-tiles
  per instruction -> 2x PE throughput, 109ns/instr measured warm).
  Weights are pre-scaled by 64 on the host so W*64 ~ N(0,1.4) sits in
  e4m3's normal range; the scale is undone exactly: tanh gets
  scale=1/64, leaky_relu is positively homogeneous so the 64x rides
  through and cancels against watt/64^2 in num and scale=1/64 inside
  the squares for den.
- Weight-tile DMAs alternate between the two HWDGE rings (sync/SP and
  scalar/ACT) so descriptor generation pipelines; consts are merged
  into single transfers.
- The causal mask is -1e30 on ALL cross-example positions, so softmax
  is one full 128x128 exp + row reduce; the 1/rowsum is folded into
  the feat PSUM->SBUF copies (rows of feat PSUM are q).
- hist for the final feat matmul and the output are bf16.
"""

import numpy as np
import ml_dtypes

import bass_rust
import concourse.bass as bass
import concourse.mybir as mybir
import concourse.tile as tile
from concourse.vector_clock import ScopedClock

# ---------------------------------------------------------------------------
# Workaround: this walrus build accepts only ONE semaphore wait on an SP
# Drain, but TileContext's tail drain carries one wait per live semaphore.
# Split them across a chain of drains.
# ---------------------------------------------------------------------------


def _patched_drain_and_barrier(self, tick_clock, wait_clock):
    nc = self.nc
    drain_inst = nc.sync.drain()
    wait_clock.add_sem_waits(
        drain_inst.ins, ScopedClock({None: tick_clock.global_clock})
    )
    waits = list(drain_inst.ins.sync_info.on_wait)
    if len(waits) > 1:
        drain_inst.ins.sync_info = bass_rust.SyncInfo(
            on_wait=waits[:1], on_update=list(drain_inst.ins.sync_info.on_update)
        )
        for i in range(1, len(waits)):
            extra = nc.sync.drain()
            extra.ins.sync_info = bass_rust.SyncInfo(
                on_wait=waits[i : i + 1], on_update=[]
            )
    nc.all_engine_barrier()
    assert self.sems is not None
    popped = nc._tile_sem_poison_stack.pop()
    assert popped is self._sem_poison
    nc.clear_and_free_semaphores(list(self.sems.allocated().values()))
    nc.all_engine_barrier()


tile.TileContext._drain_and_barrier = _patched_drain_and_barrier


def _split_multi_waits(nc):
    """This walrus build accepts at most one semaphore wait per instruction.
    Hoist extra waits onto standalone EventSemaphore instructions inserted
    just before the owning instruction in the same engine's stream."""
    uid = [0]
    for f in nc.m.functions:
        for bb in f.blocks:
            out = []
            for inst in bb.instructions:
                si = inst.sync_info
                if si is not None and len(si.on_wait) > 1:
                    waits = list(si.on_wait)
                    for w in waits[:-1]:
                        nop = mybir.InstEventSemaphore(
                            name=f"I-waitsplit-{uid[0]}", ins=[], outs=[]
                        )
                        uid[0] += 1
                        nop.engine = inst.engine
                        nop.sync_info = bass_rust.SyncInfo(
                            on_wait=[w], on_update=[]
                        )
                        out.append(nop)
                    inst.sync_info = bass_rust.SyncInfo(
                        on_wait=[waits[-1]], on_update=list(si.on_update)
                    )
                out.append(inst)
            bb.instructions[:] = out

# ---------------------------------------------------------------------------

B, R, H, IN = 64, 32, 1024, 2048
NCORES = 8
BL = B // NCORES  # examples per core
BR = BL * R  # 256 rows per core
KC = IN // 128  # 16 contraction chunks
MC = H // 128  # 8 h chunks
NEG = -1.0e30
WSCALE = 64.0  # fp8 weight pre-scale (power of two)

F32 = mybir.dt.float32
BF16 = mybir.dt.bfloat16


def build_program(mode="fp8", zero_bias=True):
    """Build the per-core Bass program. mode selects the dtype of the
    big-GEMM operands (weights + transposed activations):
    fp8 (DoubleRow, weights pre-scaled), bf16, or f32r."""
    if mode == "fp8":
        xdt = mybir.dt.float8e4
        step = 2
        pmode = mybir.MatmulPerfMode.DoubleRow
        sinv = 1.0 / WSCALE
    else:
        xdt = mybir.dt.float32r if mode == "f32r" else BF16
        step = 1
        pmode = None
        sinv = 1.0

    nc = bass.Bass()
    qt_d = nc.dram_tensor("qt", [128, KC, BR], xdt, kind="ExternalInput")
    ht_d = nc.dram_tensor("ht", [128, KC, BR], xdt, kind="ExternalInput")
    hn_d = nc.dram_tensor("hn", [128, 2, IN], BF16, kind="ExternalInput")
    wh_d = nc.dram_tensor("wh", [MC, 128, 2, KC, 128], xdt, kind="ExternalInput")
    wq_d = nc.dram_tensor("wq", [MC, 128, 2, KC, 128], xdt, kind="ExternalInput")
    # [bqy, bqg, bhy, bhg, watt] stacked -> one DMA
    bw_d = nc.dram_tensor("bw", [128, 5, MC], F32, kind="ExternalInput")
    # [mask | ident] -> one DMA
    mi_d = nc.dram_tensor("mi", [128, 256], F32, kind="ExternalInput")
    feat_d = nc.dram_tensor("feat", [2, 128, IN], BF16, kind="ExternalOutput")

    ACT = mybir.ActivationFunctionType

    with tile.TileContext(nc) as tc:
        with (
            tc.tile_pool(name="sb", bufs=1) as big,
            tc.tile_pool(name="ps", bufs=1, space="PSUM") as psp,
        ):
            sm = big
            wts = big
            tmp = big
            pse = psp
            psnd = psp
            psf = psp
            # consts: one small transfer on the scalar ring; the sync ring
            # carries the whole weight stream
            bw = sm.tile([128, 5, MC], F32, tag="bw")
            nc.scalar.dma_start(bw[:], bw_d[:])
            # ques-transposed activations: first compute dependency; split
            # across the two rings so the first matmul starts sooner
            qt = big.tile([128, KC, BR], xdt, tag="qt")
            nc.sync.dma_start(qt[:, 0:2, :], qt_d[:, 0:2, :])
            nc.scalar.dma_start(qt[:, 8:16, :], qt_d[:, 8:16, :])

            # dummy activations with no data deps: the scalar engine runs
            # them during the initial DMA wait, so the Tanh/Lrelu/Square
            # PWP table loads happen off the critical path
            ones = sm.tile([128, 1], BF16, tag="ones")
            nc.vector.memset(ones[:], 1.0)
            w0 = sm.tile([128, 1], F32, tag="w0")
            nc.scalar.activation(w0[:], bw[:, 0, 0:1], ACT.Tanh)
            nc.scalar.activation(w0[:], bw[:, 0, 0:1], ACT.Lrelu, alpha=0.01)
            nc.scalar.activation(w0[:], bw[:, 0, 0:1], ACT.Square)

            EDT = BF16
            he = big.tile([128, MC, BR], EDT, tag="he")
            he2 = big.tile([128, MC, BR], EDT, tag="he2")
            qew = big.tile([128, MC, BR], EDT, tag="qew")
            qe2 = big.tile([128, MC, BR], EDT, tag="qe2")

            num_ps = [
                psnd.tile([128, 128], F32, name=f"num{g}", tag=f"num{g}")[:]
                for g in range(2)
            ]
            den_ps = [
                psnd.tile([128, 128], F32, name=f"den{g}", tag=f"den{g}")[:]
                for g in range(2)
            ]

            def gated(xt, w_dram, iy, ig, m, split=False):
                """One contiguous y+g weight DMA (sync ring); big GEMM pair.
                Returns (ty, tg) [128, BR]: ty = tanh branch, tg = the
                (64x-scaled in fp8 mode) leaky_relu branch."""
                wt = wts.tile([128, 2, KC, 128], xdt, tag="wt", bufs=8)
                if split:
                    # head pieces first so the k=0 matmuls start after
                    # ~100KB instead of the full 512KB tile
                    nc.sync.dma_start(wt[:, 0, 0:2], w_dram[m, :, 0, 0:2])
                    nc.sync.dma_start(wt[:, 0, 2:8], w_dram[m, :, 0, 2:8])
                    nc.sync.dma_start(qt[:, 2:8, :], qt_d[:, 2:8, :])
                    nc.sync.dma_start(wt[:, 0, 8:16], w_dram[m, :, 0, 8:16])
                    nc.sync.dma_start(wt[:, 1], w_dram[m, :, 1])
                else:
                    nc.sync.dma_start(wt[:], w_dram[m])
                ps = pse.tile([128, 2 * BR], F32, tag="ps", bufs=2)
                psy, psg = ps[:, 0:BR], ps[:, BR : 2 * BR]
                for k in range(0, KC, step):
                    nc.tensor.matmul(
                        psy,
                        wt[:, 0, k : k + step, :] if step == 2 else wt[:, 0, k, :],
                        xt[:, k : k + step, :] if step == 2 else xt[:, k, :],
                        start=(k == 0), stop=(k + step == KC),
                        perf_mode=pmode,
                    )
                for k in range(0, KC, step):
                    nc.tensor.matmul(
                        psg,
                        wt[:, 1, k : k + step, :] if step == 2 else wt[:, 1, k, :],
                        xt[:, k : k + step, :] if step == 2 else xt[:, k, :],
                        start=(k == 0), stop=(k + step == KC),
                        perf_mode=pmode,
                    )
                ty = tmp.tile([128, BR], F32, tag="ty", bufs=3)
                nc.scalar.activation(
                    ty[:], psy, ACT.Tanh, bias=bw[:, iy, m : m + 1], scale=sinv
                )
                # leaky_relu(s*x) = s*leaky_relu(x): the 64x rides along
                tg = tmp.tile([128, BR], F32, tag="tg", bufs=3)
                nc.scalar.activation(
                    tg[:], psg, ACT.Lrelu, bias=bw[:, ig, m : m + 1], alpha=0.01
                )
                return ty, tg

            # ques embeddings (first: only needs qt + wq)
            for m in range(MC):
                ty, tg = gated(qt, wq_d, 0, 1, m, split=(m == 0))
                # qew = ty * (watt/64^2) * tg_scaled  -> qew_true/64
                nc.vector.scalar_tensor_tensor(
                    qew[:, m, :], ty[:], bw[:, 4, m : m + 1], tg[:],
                    op0=mybir.AluOpType.mult, op1=mybir.AluOpType.mult,
                )
                qe = tmp.tile([128, BR], F32, tag="qe", bufs=3)
                nc.vector.tensor_mul(qe[:], ty[:], tg[:])
                # (qe_scaled/64)^2 = qe_true^2
                nc.scalar.activation(qe2[:, m, :], qe[:], ACT.Square, scale=sinv)
                if m == 4:
                    # hist inputs on the scalar ring once the early rush
                    # (qt + first weight tiles) has drained
                    ht = big.tile([128, KC, BR], xdt, tag="ht")
                    nc.scalar.dma_start(ht[:], ht_d[:])
                if m == 6:
                    hn = big.tile([128, 2, IN], BF16, tag="hn")
                    nc.scalar.dma_start(hn[:], hn_d[:])
                if m == 7:
                    mi = sm.tile([128, 256], F32, tag="mi")
                    nc.scalar.dma_start(mi[:], mi_d[:])


            # hist embeddings + num/den accumulation per chunk
            for m in range(MC):
                ty, tg = gated(ht, wh_d, 2, 3, m)
                nc.vector.tensor_mul(he[:, m, :], ty[:], tg[:])
                for g in range(2):
                    sl = slice(128 * g, 128 * (g + 1))
                    nc.tensor.matmul(
                        num_ps[g], qew[:, m, sl], he[:, m, sl],
                        start=(m == 0), stop=(m == MC - 1),
                    )
                nc.scalar.activation(he2[:, m, :], he[:, m, :], ACT.Square, scale=sinv)
                for g in range(2):
                    sl = slice(128 * g, 128 * (g + 1))
                    nc.tensor.matmul(
                        den_ps[g], qe2[:, m, sl], he2[:, m, sl],
                        start=(m == 0), stop=(m == MC - 1),
                    )

            # Dummy Ln/Exp anchored on the last hist chunk's output: they
            # run right after the m=7 scalar ops, so the ~1.3us PWP table
            # loads overlap the trailing num/den matmuls instead of the
            # serial tail path below. (No-dep dummies get hoisted by the
            # Tile scheduler into the middle of the embedding phase and
    # thrash the Tanh/Lrelu/Square tables -- measured, do not.)
            warm = sm.tile([128, 1], F32, tag="warm")
            nc.scalar.activation(warm[:], he2[:, MC - 1, 0:1], ACT.Ln)
            nc.scalar.activation(warm[:], warm[:], ACT.Exp)

            # attention tail + feat: mask is -1e30 off the causal diagonal
            # blocks, so exp of the full tile zeroes cross-example terms and
            # the row sum is the softmax denominator; 1/rowsum is applied to
            # the feat PSUM rows (which are q) during the PSUM->SBUF copy.
            for g in range(2):
                # 1/sqrt(den2) = exp(-0.5*ln(den2)): two table ops on the
                # scalar engine; avoids the 950ns DVE reciprocal
                sd = tmp.tile([128, 128], F32, tag="sd", bufs=3)
                nc.scalar.activation(sd[:], den_ps[g], ACT.Ln)
                rd = tmp.tile([128, 128], F32, tag="rd", bufs=3)
                nc.scalar.activation(rd[:], sd[:], ACT.Exp, scale=-0.5)
                s = sm.tile([128, 128], F32, name=f"sc{g}", tag=f"sc{g}")
                nc.vector.tensor_mul(s[:], num_ps[g], rd[:])
                nc.vector.tensor_add(s[:], s[:], mi[:, 0:128])
                if g == 0:
                    # tiny matmul dependent on mid-chain rd: splits the PE
                    # idle window so HAM stays at full clock for the tail
                    fill = pse.tile([128, 2 * BR], F32, tag="ps", bufs=2)
                    nc.tensor.matmul(
                        fill[0:32, 0:1], rd[:, 0:32], rd[:, 0:1],
                        start=True, stop=True,
                    )
                # exp commutes with transpose: transpose the scores (PE waits
                # only ~1.3us after the last num/den matmul, keeping HAM at
                # full clock), then exp PSUM->SBUF lands att^T in bf16
                # directly -- no separate att tile or PSUM copy.
                atp = psf.tile([128, 512], F32, tag="fps", bufs=2)
                nc.tensor.transpose(atp[:, 0:128], s[:], mi[:, 128:256])
                atb = sm.tile([128, 128], BF16, name=f"atb{g}", tag=f"atb{g}")
                nc.scalar.activation(atb[:], atp[:, 0:128], ACT.Exp)
                # softmax row sums: ones-matmul over the partition dim of
                # att^T (DVE cannot reduce across partitions)
                rsp = psf.tile([128, 512], F32, tag="fps", bufs=2)
                nc.tensor.matmul(
                    rsp[:, 0:1], atb[:], ones[:], start=True, stop=True
                )
                lrs = sm.tile([128, 1], F32, name=f"lrs{g}", tag=f"lrs{g}")
                nc.scalar.activation(lrs[:], rsp[:, 0:1], ACT.Ln)
                rrs = sm.tile([128, 1], F32, name=f"rrs{g}", tag=f"rrs{g}")
                nc.scalar.activation(rrs[:], lrs[:], ACT.Exp, scale=-1.0)
                for c2 in range(2):
                    fsb = tmp.tile([128, 1024], BF16, tag="fsb", bufs=3)
                    for half in range(2):
                        c = 2 * c2 + half
                        cs = slice(512 * c, 512 * (c + 1))
                        fps = psf.tile([128, 512], F32, tag="fps", bufs=2)
                        nc.tensor.matmul(
                            fps[:], atb[:], hn[:, g, cs], start=True, stop=True
                        )
                        dst = fsb[:, 512 * half : 512 * (half + 1)]
                        if half == 0:
                            nc.scalar.activation(
                                dst, fps[:], ACT.Copy, scale=rrs[:, 0:1]
                            )
                        else:
                            nc.vector.tensor_scalar_mul(dst, fps[:], rrs[:, 0:1])
                    eng = nc.sync if c2 == 0 else nc.scalar
                    eng.dma_start(
                        feat_d[g, :, 1024 * c2 : 1024 * (c2 + 1)], fsb[:]
                    )

    _split_multi_waits(nc)
    return nc


# ---------------------------------------------------------------------------
# Host side
# ---------------------------------------------------------------------------

_PROG_CACHE = {}


def _get_prog(mode, zero_bias):
    key = (mode, zero_bias)
    if key not in _PROG_CACHE:
        _PROG_CACHE[key] = build_program(mode, zero_bias)
    return _PROG_CACHE[key]


def _xnp(mode):
    if mode == "fp8":
        return ml_dtypes.float8_e4m3
    return np.float32 if mode == "f32r" else ml_dtypes.bfloat16


def _prep_shared(W_hy, b_hy, W_hg, b_hg, W_qy, b_qy, W_qg, b_qg, W_att, mode):
    xnp = _xnp(mode)
    ws = WSCALE if mode == "fp8" else 1.0

    def reblock(W):
        # [IN, H] -> [128, MC, KC, 128]; Wr[p, m, k, h] = W[128k+p, 128m+h]
        return (W.reshape(KC, 128, MC, 128) * ws).transpose(1, 2, 0, 3).astype(xnp)

    def bvec(b):
        return np.ascontiguousarray(b.reshape(MC, 128).T).astype(np.float32)

    # causal 32x32 blocks on the diagonal, -1e30 everywhere else (kills
    # cross-example terms inside the 128-row group at exp time)
    i = np.arange(128)
    same_block = (i[:, None] // 32) == (i[None, :] // 32)
    causal = (i[None, :] % 32) <= (i[:, None] % 32)
    mask = np.where(same_block & causal, 0.0, NEG).astype(np.float32)
    # [MC, 128, 2, KC, 128]
    wh = np.ascontiguousarray(
        np.stack([reblock(W_hy), reblock(W_hg)], axis=2).transpose(1, 0, 2, 3, 4)
    )
    wq = np.ascontiguousarray(
        np.stack([reblock(W_qy), reblock(W_qg)], axis=2).transpose(1, 0, 2, 3, 4)
    )
    watt = bvec(W_att)
    if mode == "fp8":
        watt = watt / (ws * ws)
    bw = np.ascontiguousarray(
        np.stack([bvec(b_qy), bvec(b_qg), bvec(b_hy), bvec(b_hg), watt], axis=1)
    )
    mi = np.ascontiguousarray(
        np.concatenate([mask, np.eye(128, dtype=np.float32)], axis=1)
    )
    return {"wh": wh, "wq": wq, "bw": bw, "mi": mi}, xnp


def kernel(
    hist, ques, W_hy, b_hy, W_hg, b_hg, W_qy, b_qy, W_qg, b_qg, W_att, b_att,
    mode="fp8", trace=False,
):
    from concourse.bass_utils import run_bass_kernel_spmd

    hist = np.asarray(hist, np.float32)
    ques = np.asarray(ques, np.float32)
    nc = _get_prog(mode, True)
    shared, xnp = _prep_shared(
        np.asarray(W_hy, np.float32), np.asarray(b_hy, np.float32),
        np.asarray(W_hg, np.float32), np.asarray(b_hg, np.float32),
        np.asarray(W_qy, np.float32), np.asarray(b_qy, np.float32),
        np.asarray(W_qg, np.float32), np.asarray(b_qg, np.float32),
        np.asarray(W_att, np.float32), mode,
    )
    in_maps = []
    for c in range(NCORES):
        hs = hist[c * BL : (c + 1) * BL].reshape(BR, IN)
        qs = ques[c * BL : (c + 1) * BL].reshape(BR, IN)
        im = dict(shared)
        # [128, KC, BR]; qt[p, k, b] = qs[b, 128k+p]
        im["qt"] = np.ascontiguousarray(
            qs.T.reshape(KC, 128, BR).transpose(1, 0, 2)
        ).astype(xnp)
        im["ht"] = np.ascontiguousarray(
            hs.T.reshape(KC, 128, BR).transpose(1, 0, 2)
        ).astype(xnp)
        # [128, 2, IN]; hn[p, t, d] = hs[128t+p, d]
        im["hn"] = np.ascontiguousarray(
            hs.reshape(2, 128, IN).transpose(1, 0, 2)
        ).astype(ml_dtypes.bfloat16)
        in_maps.append(im)

    res = run_bass_kernel_spmd(
        nc, in_maps, core_ids=list(range(NCORES)), trace=trace
    )
    feat = np.concatenate(
        [
            r["feat"].astype(np.float32).reshape(BL, R, IN)
            for r in res.results
        ],
        axis=0,
    )
    if trace:
        return feat, res
    return feat
